# revision 28
# baseline (speedup 1.0000x reference)
"""Trainium2 Bass kernel for nn_CrossAttnVDTBlock (B=2,T=8,N=256,D=768,H=12,DFF=3072).

Sharding: 8 NeuronCores = 2 batch-groups x 4 frame-pair shards. Core c serves
batch c//4 and owns query frames (g, 7-g), g=c%4 (512 tokens, host-permuted to
the front). Collective-free: each core redundantly computes the
cross-attention stage and the self-attention K/V for its whole batch (2048
tokens), then self-attention scores/AV and the MLP only for its own 512 query
tokens. Frame-causal masks are folded into the score matmuls via 8 augmented
contraction rows (K side: one-hot frame id; Q side: -30000*[f > frame(q)]), so
masking costs no elementwise work. The host pre-fuses c_wo@w_fc1 and s_wo@w_fc2
(no nonlinearity between them), folds hd^-0.5 into wq and the V-bias into the
fused output bias (softmax rows sum to 1, so (A @ (1 x bv)) @ Wo = 1 x (bv@Wo)),
and casts weights to bf16. Matmuls run bf16 (fp32 PSUM); the residual stream
stays fp32 on-chip. Activations are feature-major [D, tokens] throughout - no
device transposes. No GPSIMD/Pool-engine ops anywhere: partition broadcasts
(LN stats, softmax denominators) go through 1-row PE matmuls into PSUM, which
avoids the multi-ms Q7 ucode library reloads that Pool custom ops trigger.
"""

import contextlib

import numpy as np
import ml_dtypes

import concourse.bass as bass
import concourse.mybir as mybir
import concourse.tile as tile
from concourse import bacc
from concourse.bass import ts
from concourse.bass_utils import run_bass_kernel_spmd

F32 = mybir.dt.float32
BF16 = mybir.dt.bfloat16
AF = mybir.ActivationFunctionType
ALU = mybir.AluOpType

B, T, NT, D, H, DFF = 2, 8, 256, 768, 12, 3072
hd = D // H          # 64
S = T * NT           # 2048
P = 128
KD = D // P          # 6 din tiles
KF = DFF // P        # 24 dff tiles
NEG = -30000.0
EPS = 1e-6
NCORE = 8
OWN = 512
NCH = S // 512       # 4 column chunks of 512

_bf = ml_dtypes.bfloat16

# Packed-input layouts: all bf16 inputs ride in one [P, WCOLS] tensor and all
# f32 inputs in one [P, FCOLS] tensor (both partition-major, so every DMA is
# one contiguous run per partition). Cuts the 33 per-dispatch input buffers
# down to 2 and slashes DMA descriptor counts.
WSEG_L = [("c_wq", KD * D), ("c_wk", KD * D), ("c_wv", KD * D),
          ("wc", KD * D), ("s_wq", KD * D), ("s_wk", KD * D),
          ("s_wv", KD * D), ("ws", KD * D), ("m_w1", KD * DFF),
          ("m_w2", KF * D), ("w_ada", KD * 6 * D), ("zb", KD * 8),
          ("tsil", KD), ("qmask", S), ("khot", S), ("zhot", 8)]
FSEG_L = [("xo", KD * OWN), ("xr", KD * (S - OWN)), ("bada", 36),
          ("cbq", KD), ("cbk", KD), ("bc", KD), ("sbq", KD), ("sbk", KD),
          ("bs", KD), ("mb2", KD), ("mb1", KF)]
WOFF, _o = {}, 0
for _n, _c in WSEG_L:
    WOFF[_n] = _o
    _o += _c
WCOLS = _o
FOFF, _o = {}, 0
for _n, _c in FSEG_L:
    FOFF[_n] = _o
    _o += _c
FCOLS = _o


def _ln(tc, nc, getx, ncols, ones_col, ones_row, sc1_ap, sh_ap, out_xt):
    """LayerNorm over features of feature-major x (via getx(j, chunk) -> AP
    [128,512]), optionally adaLN-modulated; writes bf16 out_xt [128,KD,ncols].
    Pool-free: per-token stats are broadcast across partitions with 1-row PE
    matmuls into PSUM."""
    nchunks = ncols // 512
    with tc.tile_pool(name="lnp", bufs=2, space="PSUM") as pp, \
            tc.tile_pool(name="lnb", bufs=2, space="PSUM") as pb, \
            tc.tile_pool(name="lns", bufs=2) as sp, \
            tc.tile_pool(name="lnt", bufs=3) as tp:
        for c in range(nchunks):
            ps_s = pp.tile([1, 512], F32, tag="ln_s")
            ps_q = pp.tile([1, 512], F32, tag="ln_q")
            for j in range(KD):
                xj = getx(j, c)
                xb = tp.tile([P, 512], BF16, tag="xb")
                nc.vector.tensor_copy(xb[:], xj)
                xsq = tp.tile([P, 512], BF16, tag="xsq")
                nc.scalar.activation(xsq[:], xj, AF.Square)
                nc.tensor.matmul(ps_s[:], ones_col[:], xb[:],
                                 start=(j == 0), stop=(j == KD - 1))
                nc.tensor.matmul(ps_q[:], ones_col[:], xsq[:],
                                 start=(j == 0), stop=(j == KD - 1))
            nc.vector.tensor_scalar_mul(ps_s[:], ps_s[:], -1.0 / D)
            nc.vector.tensor_scalar(ps_q[:], ps_q[:], 1.0 / D, EPS, ALU.mult,
                                    ALU.add)
            mu2 = sp.tile([1, 512], F32, tag="mu2")
            nc.scalar.activation(mu2[:], ps_s[:], AF.Square)
            nc.vector.tensor_tensor(ps_q[:], ps_q[:], mu2[:], ALU.subtract)
            nc.scalar.activation(ps_q[:], ps_q[:], AF.Sqrt)
            rr = sp.tile([1, 512], F32, tag="rr")
            nc.vector.reciprocal(rr[:], ps_q[:])
            nm = sp.tile([1, 512], F32, tag="nm")
            nc.vector.tensor_tensor(nm[:], ps_s[:], rr[:], ALU.mult)
            rrb = sp.tile([1, 512], BF16, tag="rrb")
            nc.vector.tensor_copy(rrb[:], rr[:])
            nmb = sp.tile([1, 512], BF16, tag="nmb")
            nc.vector.tensor_copy(nmb[:], nm[:])
            rbp = pb.tile([P, 512], F32, tag="rbp")
            nc.tensor.matmul(rbp[:], ones_row[:], rrb[:], start=True, stop=True)
            mbp = pb.tile([P, 512], F32, tag="mbp")
            nc.tensor.matmul(mbp[:], ones_row[:], nmb[:], start=True, stop=True)
            for j in range(KD):
                cs = ts(c, 512)
                t1 = tp.tile([P, 512], F32, tag="lnt1")
                nc.vector.tensor_tensor(t1[:], getx(j, c), rbp[:], ALU.mult)
                if sc1_ap is None:
                    nc.vector.tensor_tensor(out_xt[:, j, cs], t1[:], mbp[:],
                                            ALU.add)
                else:
                    nc.vector.tensor_tensor(t1[:], t1[:], mbp[:], ALU.add)
                    nc.vector.tensor_scalar(out_xt[:, j, cs], t1[:],
                                            sc1_ap[:, j, None],
                                            sh_ap[:, j, None],
                                            ALU.mult, ALU.add)


def _emit_kernel(tc, io, stages="full"):
    nc = tc.nc
    st = contextlib.ExitStack()
    pool = lambda **kw: st.enter_context(tc.tile_pool(**kw))

    def _done():
        nc.sync.dma_start(io["xout"],
                          x_own[:])
        st.close()

    persist = pool(name="persist", bufs=1)
    tmp = pool(name="tmp", bufs=3)
    small = pool(name="small", bufs=3)

    # ---------------- persistent state ----------------
    x_own = persist.tile([P, KD, OWN], F32, tag="x_own")
    ones_col = persist.tile([P, 1], BF16, tag="ones_c")
    nc.vector.memset(ones_col[:], 1.0)
    ones_row = persist.tile([1, P], BF16, tag="ones_r")
    nc.vector.memset(ones_row[:], 1.0)
    mods = persist.tile([P, 36], F32, tag="mods")
    qmask = persist.tile([8, S], BF16, tag="qmask")
    khot = persist.tile([8, S], BF16, tag="khot")
    zhot = persist.tile([8, 8], BF16, tag="zhot")
    zb = persist.tile([P, KD, 8], BF16, tag="zb")
    u2 = persist.tile([P, KD, OWN], BF16, tag="u2")

    nc.sync.dma_start(x_own[:],
                      io["xT_own"])
    nc.sync.dma_start(qmask[:], io["qmask"][:])
    nc.sync.dma_start(khot[:], io["khot"][:])
    nc.sync.dma_start(zhot[:], io["zhot"][:])
    nc.sync.dma_start(zb[:], io["zb"])

    bias = {}
    for nm_ in ("cbq", "cbk", "bc", "sbq", "sbk", "bs", "mb2"):
        bt = persist.tile([P, KD], F32, tag="b_" + nm_)
        nc.sync.dma_start(bt[:], io[nm_][:])
        bias[nm_] = bt
    mb1 = persist.tile([P, KF], F32, tag="b_mb1")
    nc.sync.dma_start(mb1[:], io["mb1"][:])

    def modap(i):  # chunk i of (sh_s, sc_s, g_s, sh_m, sc_m, g_m)
        return mods[:, ts(i, KD)]

    # ---------------- mods = silu(t) @ w_ada + b_ada ----------------
    with tc.tile_pool(name="ada", bufs=1) as ap, \
            tc.tile_pool(name="adap", bufs=1, space="PSUM") as app:
        wada = ap.tile([P, KD, 6 * D], BF16)
        nc.sync.dma_start(wada[:],
                          io["w_ada"])
        tsil = ap.tile([P, KD], BF16)
        nc.sync.dma_start(tsil[:], io["tsil"][:])
        tsig = ap.tile([P, KD], BF16)
        nc.scalar.activation(tsig[:], tsil[:], AF.Silu)
        bada = ap.tile([P, 36], F32)
        nc.sync.dma_start(bada[:], io["bada"][:])
        ps_m = app.tile([P, 36], F32)
        for m in range(36):
            for j in range(KD):
                nc.tensor.matmul(ps_m[:, m:m + 1], wada[:, j, ts(m, P)],
                                 tsig[:, j, None], start=(j == 0),
                                 stop=(j == KD - 1))
        nc.vector.tensor_tensor(mods[:], ps_m[:], bada[:], ALU.add)

    if stages == "ada":
        _done()
        return

    # =========== stages 1+2 need the full-batch residual ===========
    with tc.tile_pool(name="bigx", bufs=1) as bigp:
        xst = contextlib.ExitStack()
        xrp = xst.enter_context(tc.tile_pool(name="xrestp", bufs=1))
        x_rest = xrp.tile([P, KD, S - OWN], F32, tag="x_rest")
        nc.sync.dma_start(
            x_rest[:], io["xT_rest"])
        xt = bigp.tile([P, KD, S], BF16, tag="xt")  # normalized activations

        def getx(j, c):
            if c == 0:
                return x_own[:, j, :]
            return x_rest[:, j, ts(c - 1, 512)]

        # ---------------- stage 1: cross attention ----------------
        _ln(tc, nc, getx, S, ones_col, ones_row, None, None, xt)

        if stages == "ln1":
            xst.close()
            _done()
            return

        with tc.tile_pool(name="s1w", bufs=2) as wp, \
                tc.tile_pool(name="s1", bufs=1) as s1p, \
                tc.tile_pool(name="s1q", bufs=2) as qcp:
            wk = wp.tile([P, KD, D], BF16, tag="w")
            nc.sync.dma_start(wk[:],
                              io["c_wk"])
            wv = wp.tile([P, KD, D], BF16, tag="w")
            nc.sync.dma_start(wv[:],
                              io["c_wv"])
            # kz feature-major [72, 8] per head; vz grouped [8, 12, 65]
            kz = s1p.tile([72, 8 * H], BF16, tag="kz")
            vz = s1p.tile([8, H, 65], BF16, tag="vz")
            with tc.tile_pool(name="s1prep", bufs=2, space="PSUM") as pprep:
                for j in range(KD):
                    ps = pprep.tile([P, 8], F32, tag="proj8")
                    for k in range(KD):
                        nc.tensor.matmul(ps[:], wk[:, k, ts(j, P)], zb[:, k, :],
                                         start=(k == 0), stop=(k == KD - 1))
                    for hh in (2 * j, 2 * j + 1):
                        r0 = (hh % 2) * 64
                        nc.scalar.activation(
                            kz[0:64, ts(hh, 8)], ps[r0:r0 + 64, :], AF.Identity,
                            bias=bias["cbk"][r0:r0 + 64, j, None])
                for hh in range(H):
                    nc.vector.tensor_copy(kz[64:72, ts(hh, 8)], zhot[:])
                for ck, cw in ((0, 512), (512, 256)):
                    ps = pprep.tile([8, 512], F32, tag="projz")
                    for k in range(KD):
                        nc.tensor.matmul(ps[:, 0:cw], zb[:, k, :],
                                         wv[:, k, ck:ck + cw], start=(k == 0),
                                         stop=(k == KD - 1))
                    h0, nh = ck // 64, cw // 64
                    nc.vector.tensor_copy(
                        vz[:, h0:h0 + nh, 0:64],
                        ps[:, 0:cw].rearrange("p (h d) -> p h d", d=64))
                nc.vector.memset(vz[:, :, 64:65], 1.0)

            wq = wp.tile([P, KD, D], BF16, tag="w")
            nc.sync.dma_start(wq[:],
                              io["c_wq"])
            u1 = s1p.tile([P, KD, S], BF16, tag="u1")
            s1ctx = contextlib.ExitStack()
            pmm = s1ctx.enter_context(
                tc.tile_pool(name="s1mm", bufs=2, space="PSUM"))
            psc = s1ctx.enter_context(
                tc.tile_pool(name="s1sc", bufs=2, space="PSUM"))
            pav = s1ctx.enter_context(
                tc.tile_pool(name="s1av", bufs=2, space="PSUM"))
            pdb = s1ctx.enter_context(
                tc.tile_pool(name="s1db", bufs=2, space="PSUM"))
            for j in range(KD):
                qa = {}
                for hh in (2 * j, 2 * j + 1):
                    qa[hh] = qcp.tile([72, S], BF16, tag="qc", name=f"qc{j}_{hh}")
                    nc.vector.tensor_copy(qa[hh][64:72, :], qmask[:])
                for c in range(NCH):
                    ps = pmm.tile([P, 512], F32, tag="proj")
                    for k in range(KD):
                        nc.tensor.matmul(ps[:], wq[:, k, ts(j, P)],
                                         xt[:, k, ts(c, 512)], start=(k == 0),
                                         stop=(k == KD - 1))
                    for hh in (2 * j, 2 * j + 1):
                        r0 = (hh % 2) * 64
                        nc.scalar.activation(
                            qa[hh][0:64, ts(c, 512)], ps[r0:r0 + 64, :],
                            AF.Identity,
                            bias=bias["cbq"][r0:r0 + 64, j, None])
                for hh in (2 * j, 2 * j + 1):
                    for c in range(NCH):
                        ps = psc.tile([8, 512], F32, tag="zsc")
                        nc.tensor.matmul(ps[:], kz[:, ts(hh, 8)],
                                         qa[hh][:, ts(c, 512)],
                                         start=True, stop=True)
                        e8 = tmp.tile([8, 512], BF16, tag="e8")
                        nc.scalar.activation(e8[:], ps[:], AF.Exp)
                        ov = pav.tile([65, 512], F32, tag="zav")
                        nc.tensor.matmul(ov[:], vz[:, hh, :], e8[:],
                                         start=True, stop=True)
                        den = small.tile([1, 512], F32, tag="den1")
                        nc.vector.reciprocal(den[:], ov[64:65, :])
                        denb = small.tile([1, 512], BF16, tag="denb1")
                        nc.vector.tensor_copy(denb[:], den[:])
                        db = pdb.tile([64, 512], F32, tag="db")
                        nc.tensor.matmul(db[:], ones_row[0:1, 0:64], denb[:],
                                         start=True, stop=True)
                        dbs = tmp.tile([64, 512], F32, tag="dbs")
                        nc.scalar.activation(dbs[:], db[:], AF.Identity)
                        nc.vector.tensor_tensor(
                            u1[(hh % 2) * 64:(hh % 2) * 64 + 64, hh // 2,
                               ts(c, 512)],
                            ov[0:64, :], dbs[:], ALU.mult)

            wc = wp.tile([P, KD, D], BF16, tag="w")
            nc.sync.dma_start(wc[:],
                              io["wc"])
            for j in range(KD):
                for c in range(NCH):
                    ps = pmm.tile([P, 512], F32, tag="proj")
                    for k in range(KD):
                        nc.tensor.matmul(ps[:], wc[:, k, ts(j, P)],
                                         u1[:, k, ts(c, 512)], start=(k == 0),
                                         stop=(k == KD - 1))
                    up = tmp.tile([P, 512], F32, tag="upd")
                    nc.scalar.activation(up[:], ps[:], AF.Identity,
                                         bias=bias["bc"][:, j, None])
                    dst = getx(j, c)
                    nc.vector.tensor_tensor(dst, dst, up[:], ALU.add)
            s1ctx.close()

        if stages == "s1":
            xst.close()
            _done()
            return

        # ---------------- stage 2: self attention ----------------
        sc1_s = persist.tile([P, KD], F32, tag="sc1_s")
        nc.vector.tensor_scalar(sc1_s[:], modap(1), 1.0, None, ALU.add)
        _ln(tc, nc, getx, S, ones_col, ones_row, sc1_s, modap(0), xt)
        xst.close()  # x_rest dead: free 36KB/partition before attention

        with tc.tile_pool(name="s2w", bufs=2) as wp, \
                tc.tile_pool(name="s2", bufs=1) as s2p, \
                tc.tile_pool(name="s2k", bufs=3) as kqp, \
                tc.tile_pool(name="s2mm", bufs=2, space="PSUM") as pmm, \
                tc.tile_pool(name="s2sc", bufs=3, space="PSUM") as psc, \
                tc.tile_pool(name="s2av", bufs=2, space="PSUM") as pav, \
                tc.tile_pool(name="s2db", bufs=1, space="PSUM") as pdb:
            wv2 = wp.tile([P, KD, D], BF16, tag="w")
            nc.sync.dma_start(wv2[:],
                              io["s_wv"])
            vpad = s2p.tile([P, S // P, H * 65], BF16, tag="vpad")
            for i in range(S // P):
                for ck, cw in ((0, 512), (512, 256)):
                    ps = pmm.tile([P, 512], F32, tag="proj")
                    for k in range(KD):
                        nc.tensor.matmul(
                            ps[:, 0:cw],
                            xt[:, k, ts(i, P)], wv2[:, k, ck:ck + cw],
                            start=(k == 0), stop=(k == KD - 1))
                    h0, nh = ck // 64, cw // 64
                    nc.vector.tensor_copy(
                        vpad[:, i, 65 * h0:65 * (h0 + nh)].rearrange(
                            "p (h d) -> p h d", d=65)[:, :, 0:64],
                        ps[:, 0:cw].rearrange("p (h d) -> p h d", d=64))
            nc.vector.memset(
                vpad[:].rearrange("p i (h d) -> p i h d", d=65)[:, :, :,
                                                                64:65], 1.0)

            wq2 = wp.tile([P, KD, D], BF16, tag="w")
            nc.sync.dma_start(wq2[:],
                              io["s_wq"])
            wk2 = wp.tile([P, KD, D], BF16, tag="w")
            nc.sync.dma_start(wk2[:],
                              io["s_wk"])
            for j in range(KD):
                kpa, qa = {}, {}
                for hh in (2 * j, 2 * j + 1):
                    kpa[hh] = kqp.tile([72, S], BF16, tag="kpad", name=f"kp{j}_{hh}")
                    nc.vector.tensor_copy(kpa[hh][64:72, :], khot[:])
                    qa[hh] = kqp.tile([72, OWN], BF16, tag="q2a", name=f"q2{j}_{hh}")
                    nc.vector.tensor_copy(qa[hh][64:72, :], qmask[:, 0:OWN])
                for c in range(NCH):
                    ps = pmm.tile([P, 512], F32, tag="proj")
                    for k in range(KD):
                        nc.tensor.matmul(ps[:], wk2[:, k, ts(j, P)],
                                         xt[:, k, ts(c, 512)], start=(k == 0),
                                         stop=(k == KD - 1))
                    for hh in (2 * j, 2 * j + 1):
                        r0 = (hh % 2) * 64
                        nc.scalar.activation(
                            kpa[hh][0:64, ts(c, 512)], ps[r0:r0 + 64, :],
                            AF.Identity,
                            bias=bias["sbk"][r0:r0 + 64, j, None])
                ps = pmm.tile([P, 512], F32, tag="proj")
                for k in range(KD):
                    nc.tensor.matmul(ps[:], wq2[:, k, ts(j, P)],
                                     xt[:, k, 0:OWN], start=(k == 0),
                                     stop=(k == KD - 1))
                for hh in (2 * j, 2 * j + 1):
                    r0 = (hh % 2) * 64
                    nc.scalar.activation(qa[hh][0:64, :], ps[r0:r0 + 64, :],
                                         AF.Identity,
                                         bias=bias["sbq"][r0:r0 + 64, j, None])
                # Prefix-K: query half A (own frame g<=3) only attends
                # frames <= 3, which in perm order live in ktiles
                # {0,1} u {4..9}; half B (frame 7-g) needs all 16. The aug
                # rows still mask the overreach exactly.
                A_KT = [0, 1, 4, 5, 6, 7, 8, 9]
                for hh in (2 * j, 2 * j + 1):
                    ov = pav.tile([65, OWN], F32, tag="av")
                    for half, kts in ((0, A_KT), (1, list(range(16)))):
                        qs = ts(half, 256)
                        n = len(kts)
                        for pp in range(n // 2):
                            ps2 = psc.tile([P, 2, 256], F32, tag="sc")
                            for i in range(2):
                                kt = kts[pp * 2 + i]
                                nc.tensor.matmul(ps2[:, i, :],
                                                 kpa[hh][:, ts(kt, P)],
                                                 qa[hh][:, qs], start=True,
                                                 stop=True)
                            e = tmp.tile([P, 2, 256], BF16, tag="e")
                            nc.scalar.activation(e[:], ps2[:], AF.Exp)
                            for i in range(2):
                                kt = kts[pp * 2 + i]
                                nc.tensor.matmul(
                                    ov[:, qs], vpad[:, kt, ts(hh, 65)],
                                    e[:, i, :],
                                    start=(pp == 0 and i == 0),
                                    stop=(pp == n // 2 - 1 and i == 1))
                    den = small.tile([1, OWN], F32, tag="den2")
                    nc.vector.reciprocal(den[:], ov[64:65, :])
                    denb = small.tile([1, OWN], BF16, tag="denb2")
                    nc.vector.tensor_copy(denb[:], den[:])
                    db = pdb.tile([64, OWN], F32, tag="db2")
                    nc.tensor.matmul(db[:], ones_row[0:1, 0:64], denb[:],
                                     start=True, stop=True)
                    dbs = tmp.tile([64, OWN], F32, tag="dbs2")
                    nc.scalar.activation(dbs[:], db[:], AF.Identity)
                    nc.vector.tensor_tensor(
                        u2[(hh % 2) * 64:(hh % 2) * 64 + 64, hh // 2, :],
                        ov[0:64, :], dbs[:], ALU.mult)

            ws = wp.tile([P, KD, D], BF16, tag="w")
            nc.sync.dma_start(ws[:],
                              io["ws"])
            for j in range(KD):
                ps = pmm.tile([P, 512], F32, tag="proj")
                for k in range(KD):
                    nc.tensor.matmul(ps[:], ws[:, k, ts(j, P)], u2[:, k, :],
                                     start=(k == 0), stop=(k == KD - 1))
                up = tmp.tile([P, OWN], F32, tag="upd")
                nc.vector.tensor_scalar(up[:], ps[:], bias["bs"][:, j, None],
                                        modap(2)[:, j, None], ALU.add,
                                        ALU.mult)
                nc.vector.tensor_tensor(x_own[:, j, :], x_own[:, j, :], up[:],
                                        ALU.add)

    if stages == "s2":
        _done()
        return

    # ---------------- stage 3: MLP (own tokens) ----------------
    sc1_m = persist.tile([P, KD], F32, tag="sc1_m")
    nc.vector.tensor_scalar(sc1_m[:], modap(4), 1.0, None, ALU.add)
    with tc.tile_pool(name="mlp", bufs=1) as mp:
        x3 = mp.tile([P, KD, OWN], BF16, tag="x3")
        _ln(tc, nc, lambda j, c: x_own[:, j, :], OWN, ones_col, ones_row,
            sc1_m, modap(3), x3)
        mlpctx = contextlib.ExitStack()
        pmm = mlpctx.enter_context(
            tc.tile_pool(name="mmm", bufs=3, space="PSUM"))
        w1 = mp.tile([P, KD, DFF], BF16, tag="w1")
        nc.sync.dma_start(w1[:], io["m_w1"])
        h1 = mp.tile([P, KF, OWN], BF16, tag="h1")
        for j in range(KF):
            ps = pmm.tile([P, OWN], F32, tag="proj")
            for k in range(KD):
                nc.tensor.matmul(ps[:], w1[:, k, ts(j, P)], x3[:, k, :],
                                 start=(k == 0), stop=(k == KD - 1))
            nc.scalar.activation(h1[:, j, :], ps[:], AF.Gelu_apprx_tanh,
                                 bias=mb1[:, j, None])
        w2 = mp.tile([P, KF, D], BF16, tag="w2")
        nc.sync.dma_start(w2[:], io["m_w2"])
        for j in range(KD):
            ps = pmm.tile([P, OWN], F32, tag="proj")
            for k in range(KF):
                nc.tensor.matmul(ps[:], w2[:, k, ts(j, P)], h1[:, k, :],
                                 start=(k == 0), stop=(k == KF - 1))
            up = tmp.tile([P, OWN], F32, tag="upd")
            nc.vector.tensor_scalar(up[:], ps[:], bias["mb2"][:, j, None],
                                    modap(5)[:, j, None], ALU.add, ALU.mult)
            nc.vector.tensor_tensor(x_own[:, j, :], x_own[:, j, :], up[:],
                                    ALU.add)
        mlpctx.close()

    nc.sync.dma_start(io["xout"],
                      x_own[:])
    st.close()


def _build_nc(stages="full"):
    nc = bacc.Bacc("TRN2", target_bir_lowering=False, debug=False,
                   num_devices=NCORE)
    wpack = nc.dram_tensor("wpack", [P, WCOLS], BF16,
                           kind="ExternalInput").ap()
    fpack = nc.dram_tensor("fpack", [P, FCOLS], F32,
                           kind="ExternalInput").ap()

    def wseg(name, cols):
        return wpack[:, WOFF[name]:WOFF[name] + cols]

    def fseg(name, cols):
        return fpack[:, FOFF[name]:FOFF[name] + cols]

    io = {}
    io["xT_own"] = fseg("xo", KD * OWN).rearrange("p (j t) -> p j t", t=OWN)
    io["xT_rest"] = fseg("xr", KD * (S - OWN)).rearrange(
        "p (j t) -> p j t", t=S - OWN)
    io["zb"] = wseg("zb", KD * 8).rearrange("p (j t) -> p j t", t=8)
    io["tsil"] = wseg("tsil", KD)
    io["bada"] = fseg("bada", 36)
    for b in ("cbq", "cbk", "bc", "sbq", "sbk", "bs", "mb2"):
        io[b] = fseg(b, KD)
    io["mb1"] = fseg("mb1", KF)
    io["qmask"] = wseg("qmask", S)[0:8, :]
    io["khot"] = wseg("khot", S)[0:8, :]
    io["zhot"] = wseg("zhot", 8)[0:8, :]
    for w in ("c_wq", "c_wk", "c_wv", "wc", "s_wq", "s_wk", "s_wv", "ws"):
        io[w] = wseg(w, KD * D).rearrange("p (j o) -> p j o", o=D)
    io["m_w1"] = wseg("m_w1", KD * DFF).rearrange("p (j o) -> p j o", o=DFF)
    io["m_w2"] = wseg("m_w2", KF * D).rearrange("p (j o) -> p j o", o=D)
    io["w_ada"] = wseg("w_ada", KD * 6 * D).rearrange(
        "p (j o) -> p j o", o=6 * D)
    io["xout"] = nc.dram_tensor("xout", [P, KD, OWN], F32,
                                kind="ExternalOutput").ap()

    with tile.TileContext(nc) as tc:
        _emit_kernel(tc, io, stages=stages)
    nc.compile()
    return nc


_NC_CACHE = {}
LAST_RESULTS = {}


def host_prep(inputs):
    ip = {k: np.asarray(v, np.float32) for k, v in inputs.items()
          if k != "n_frames"}
    sc = hd ** -0.5
    w = {}
    w["c_wq"] = (ip["c_wq"] * sc).astype(_bf)
    w["cbq_f"] = ip["c_bq"] * sc
    w["c_wk"] = ip["c_wk"].astype(_bf)
    w["cbk_f"] = ip["c_bk"]
    w["c_wv"] = ip["c_wv"].astype(_bf)
    wc_f = ip["c_wo"] @ ip["w_fc1"]
    w["wc"] = wc_f.astype(_bf)
    w["bc_f"] = ip["c_bv"] @ wc_f + ip["c_bo"] @ ip["w_fc1"] + ip["b_fc1"]
    w["s_wq"] = (ip["s_wq"] * sc).astype(_bf)
    w["sbq_f"] = ip["s_bq"] * sc
    w["s_wk"] = ip["s_wk"].astype(_bf)
    w["sbk_f"] = ip["s_bk"]
    w["s_wv"] = ip["s_wv"].astype(_bf)
    ws_f = ip["s_wo"] @ ip["w_fc2"]
    w["ws"] = ws_f.astype(_bf)
    w["bs_f"] = ip["s_bv"] @ ws_f + ip["s_bo"] @ ip["w_fc2"] + ip["b_fc2"]
    w["m_w1"] = ip["m_w1"].astype(_bf)
    w["mb1_f"] = ip["m_b1"]
    w["m_w2"] = ip["m_w2"].astype(_bf)
    w["mb2_f"] = ip["m_b2"]
    w["w_ada"] = ip["w_ada"].astype(_bf)
    w["bada_f"] = ip["b_ada"]
    return ip, w


def _ftile(v):
    """[n*128] -> [128, n] feature-tile layout (partition p, tile j) = v[128j+p]."""
    return np.ascontiguousarray(v.reshape(-1, P).T).astype(np.float32)


def _pack_rows(v, O):
    """[n*128, O] -> [128, n*O]: row j*128+p lands at [p, j*O:(j+1)*O]."""
    return np.ascontiguousarray(
        np.asarray(v).reshape(-1, P, O).transpose(1, 0, 2).reshape(P, -1))


def core_in_map(c, ip, w):
    g, b = c % 4, c // 4
    fA, fB = g, 7 - g
    perm = [fA, fB] + [f for f in range(8) if f not in (fA, fB)]
    x = ip["x"]
    x_perm = np.concatenate([x[b * T + fr] for fr in perm], axis=0)
    frame_of = np.repeat(np.array(perm), NT)
    qmask = np.where(np.arange(8)[:, None] > frame_of[None, :], NEG,
                     0.0).astype(_bf)
    khot = (frame_of[None, :] == np.arange(8)[:, None]).astype(_bf)

    wp = np.zeros((P, WCOLS), _bf)

    def putw(name, arr):
        off = WOFF[name]
        wp[:arr.shape[0], off:off + arr.shape[1]] = arr.astype(_bf)

    for nm_ in ("c_wq", "c_wk", "c_wv", "wc", "s_wq", "s_wk", "s_wv", "ws"):
        putw(nm_, _pack_rows(w[nm_], D))
    putw("m_w1", _pack_rows(w["m_w1"], DFF))
    putw("m_w2", _pack_rows(w["m_w2"], D))
    putw("w_ada", _pack_rows(w["w_ada"], 6 * D))
    putw("zb", _pack_rows(np.ascontiguousarray(ip["z"][b].T).astype(_bf), 8))
    putw("tsil", _ftile(ip["t"][b]).astype(_bf))
    putw("qmask", qmask)
    putw("khot", khot)
    putw("zhot", np.eye(8, dtype=np.float32).astype(_bf))

    fp = np.zeros((P, FCOLS), np.float32)

    def putf(name, arr):
        off = FOFF[name]
        fp[:arr.shape[0], off:off + arr.shape[1]] = arr.astype(np.float32)

    xT = np.ascontiguousarray(x_perm.T)
    putf("xo", _pack_rows(xT[:, 0:OWN], OWN))
    putf("xr", _pack_rows(xT[:, OWN:S], S - OWN))
    putf("bada", _ftile(w["bada_f"]).reshape(P, 36))
    putf("cbq", _ftile(w["cbq_f"]))
    putf("cbk", _ftile(w["cbk_f"]))
    putf("bc", _ftile(w["bc_f"]))
    putf("sbq", _ftile(w["sbq_f"]))
    putf("sbk", _ftile(w["sbk_f"]))
    putf("bs", _ftile(w["bs_f"]))
    putf("mb2", _ftile(w["mb2_f"]))
    putf("mb1", _ftile(w["mb1_f"]))
    return {"wpack": wp, "fpack": fp}


def kernel(**inputs):
    import os
    try:
        from antenv.axon_hooks import get_axon_ntff_profile_hook  # noqa: F401
    except Exception:
        # BASS_TRACE without the axon NTFF hook raises inside
        # run_bass_kernel_spmd; force the non-trace path in that case.
        os.environ.setdefault("BASS_NEVER_TRACE", "1")
    ip, w = host_prep(inputs)
    in_maps = [core_in_map(c, ip, w) for c in range(NCORE)]
    if "nc" not in _NC_CACHE:
        _NC_CACHE["nc"] = _build_nc()
    nc = _NC_CACHE["nc"]
    res = run_bass_kernel_spmd(nc, in_maps, core_ids=list(range(NCORE)))
    LAST_RESULTS["res"] = res
    out = np.zeros((B * T, NT, D), np.float32)
    for c in range(NCORE):
        g, b = c % 4, c // 4
        fA, fB = g, 7 - g
        xo = np.asarray(res.results[c]["xout"]).transpose(1, 0, 2).reshape(
            D, OWN)
        out[b * T + fA] = xo[:, :NT].T
        out[b * T + fB] = xo[:, NT:2 * NT].T
    return out
